# revision 72
# baseline (speedup 1.0000x reference)
"""Trainium2 Bass kernel for a local-attention transformer block (v3, fp16).

Computes, per batch element (one NeuronCore each, 8 cores):
  ss = silu(t_emb) @ time_w + time_b ;  scale, shift = split(ss)
  y  = LN(x) * (1+scale) + shift                       (ln1 g/b host-folded)
  q,k,v = y @ qkv_w + qkv_b  (heads=8, d=64)
  attn: each 128-token window attends to [prev|cur|next] windows
  x1 = x + attn @ proj_w + proj_b
  out = x1 + gelu(LN2(x1) @ w1 + b1') @ w2 + b2        (ln2 g/b folded into w1/b1)

Strategy (evolved from a 2.18 ms fp16 baseline to ~1.59 ms; engine queues
execute strictly in scheduled order, so emission interleaving is the main
overlap lever):
  - All GEMMs fp16 (measured: fp16=bf16=fp8 all stream 216 ns per N=512
    matmul, and DoubleRow's 256-col LDWEIGHTS doesn't background-load, so
    fp8 gains nothing). Weights stored [128, n_chunks, out] fp16.
  - Attention key-block-major: per (head, key block j) ONE sim matmul of
    N<=384 (q windows j-1..j+1, keys on partitions), exp into an E tile
    reused by 3 AV windows; AV accumulates [65, 4win, 128] PSUM per head
    (ones column folded into v_aug produces softmax denominators).
  - Softmax normalization fully on-chip: per-head reciprocal_approx_fast of
    the PSUM sums row, bf16 cast, PE ones-outer-product broadcast across the
    head's 64 feature partitions, DVE multiply (no DRAM round trip).
  - Next group's QKV matmuls are emitted as 12 "pieces" interleaved between
    attention steps, filling PE queue bubbles while exps run on ACT; LN1 runs
    two groups ahead so its ACT sqrt isn't queued behind attention exps.
  - LN transposes x_hat fp16 via ONE batched DMA transpose per token tile
    ([128,512] -> [128,4,128], same 1.2 us as a 128x128 transpose); LN1
    modulate fused into a per-chunk tensor_scalar on the transposed side.
    xh transposes on the sync ring, h2 transposes on the scalar ring (a
    tensor's transposes must stay on ONE ring: splitting one tensor across
    rings corrupts data via the shared xbar mode).
  - ACT runs Exp/Gelu/Sqrt plus the odd-head PSUM evacuations (DVE cannot
    write cross-partition except at 32-multiples; DVE covers aligned ones).
"""

import numpy as np
from contextlib import ExitStack

import concourse.bass as bass
import concourse.tile as tile
from concourse import bacc, mybir
from concourse import bass_utils

F32 = mybir.dt.float32
F16 = mybir.dt.float16
BF16 = mybir.dt.bfloat16
AF = mybir.ActivationFunctionType
AL = mybir.AluOpType

DIM = 512
HEADS = 8
HD = 64
FF = 2048
WIN = 128
B = 8
NTOK = 8192
EPS = 1e-5
GRP = 512  # tokens per group (4 windows)
SIMSCALE = float(HD) ** -0.5


def _col_view(dram_ap, offset, ncol):
    """AP reading dram vector [128*ncol] as [128, ncol] feature-major columns."""
    return bass.AP(tensor=dram_ap.tensor, offset=offset, ap=[[1, 128], [128, ncol]])


def _bcast_row(dram_ap, offset, n):
    """AP reading dram vector [n] broadcast across 128 partitions."""
    return bass.AP(tensor=dram_ap.tensor, offset=offset, ap=[[0, 128], [1, n]])


def build(n_tok=NTOK):
    n_groups = n_tok // GRP
    nW = n_tok // WIN
    nc = bacc.Bacc("TRN2", target_bir_lowering=False, debug=False)

    x_d = nc.dram_tensor("x", [n_tok, DIM], F32, kind="ExternalInput")
    arow_d = nc.dram_tensor("arow", [DIM], F32, kind="ExternalInput")
    crow_d = nc.dram_tensor("crow", [DIM], F32, kind="ExternalInput")
    qkvw_d = nc.dram_tensor("qkvw", [128, 4, 3 * DIM], F16, kind="ExternalInput")
    qkb_d = nc.dram_tensor("qkb", [2 * DIM], F32, kind="ExternalInput")
    vb_d = nc.dram_tensor("vb", [DIM], F32, kind="ExternalInput")
    projw_d = nc.dram_tensor("projw", [128, 4, DIM], F16, kind="ExternalInput")
    projb_d = nc.dram_tensor("projb", [DIM], F32, kind="ExternalInput")
    w1_d = nc.dram_tensor("w1", [128, 4, FF], F16, kind="ExternalInput")
    b1_d = nc.dram_tensor("b1", [FF], F32, kind="ExternalInput")
    w2_d = nc.dram_tensor("w2", [128, 16, DIM], F16, kind="ExternalInput")
    b2_d = nc.dram_tensor("b2", [DIM], F32, kind="ExternalInput")
    out_d = nc.dram_tensor("out", [n_tok, DIM], F32, kind="ExternalOutput")

    with tile.TileContext(nc) as tc:
        with ExitStack() as ctx:
            consts = ctx.enter_context(tc.tile_pool(name="consts", bufs=1))
            xp = ctx.enter_context(tc.tile_pool(name="xp", bufs=3))
            xpbp = ctx.enter_context(tc.tile_pool(name="xpbp", bufs=3))
            xhp = ctx.enter_context(tc.tile_pool(name="xhp", bufs=1))
            ytp = ctx.enter_context(tc.tile_pool(name="ytp", bufs=2))
            xhtp = ctx.enter_context(tc.tile_pool(name="xhtp", bufs=1))
            qp = ctx.enter_context(tc.tile_pool(name="qp", bufs=2))
            kp = ctx.enter_context(tc.tile_pool(name="kp", bufs=2))
            vp = ctx.enter_context(tc.tile_pool(name="vp", bufs=2))
            ep = ctx.enter_context(tc.tile_pool(name="ep", bufs=1))
            astp = ctx.enter_context(tc.tile_pool(name="astp", bufs=1))
            a16p = ctx.enter_context(tc.tile_pool(name="a16p", bufs=1))
            x1p = ctx.enter_context(tc.tile_pool(name="x1p", bufs=1))
            h2tp = ctx.enter_context(tc.tile_pool(name="h2tp", bufs=1))
            gelp = ctx.enter_context(tc.tile_pool(name="gelp", bufs=1))
            op = ctx.enter_context(tc.tile_pool(name="op", bufs=2))
            sp = ctx.enter_context(tc.tile_pool(name="sp", bufs=2))
            tp = ctx.enter_context(tc.tile_pool(name="tp", bufs=2))
            ps_g = ctx.enter_context(tc.tile_pool(name="ps_g", bufs=3, space="PSUM"))
            ps_s = ctx.enter_context(tc.tile_pool(name="ps_s", bufs=3, space="PSUM"))
            ps_a = ctx.enter_context(tc.tile_pool(name="ps_a", bufs=2, space="PSUM"))

            # ---- constants ----
            qkvw_sb = consts.tile([128, 4, 3 * DIM], F16, name="qkvw_sb")
            nc.sync.dma_start(qkvw_sb[:], qkvw_d[:, :, :])
            projw_sb = consts.tile([128, 4, DIM], F16, name="projw_sb")
            nc.sync.dma_start(projw_sb[:], projw_d[:, :, :])
            w1_sb = consts.tile([128, 4, FF], F16, name="w1_sb")
            nc.sync.dma_start(w1_sb[:], w1_d[:, :, :])
            w2_sb = consts.tile([128, 16, DIM], F16, name="w2_sb")
            nc.sync.dma_start(w2_sb[:], w2_d[:, :, :])

            arow_col = consts.tile([128, 4], F32, name="arow_col")
            nc.sync.dma_start(arow_col[:], _col_view(arow_d.ap(), 0, 4))
            crow_col = consts.tile([128, 4], F32, name="crow_col")
            nc.sync.dma_start(crow_col[:], _col_view(crow_d.ap(), 0, 4))
            qkb_sb = consts.tile([128, 8], F32, name="qkb_sb")
            nc.sync.dma_start(qkb_sb[:], _col_view(qkb_d.ap(), 0, 8))
            b1_sb = consts.tile([128, 16], F32, name="b1_sb")
            nc.sync.dma_start(b1_sb[:], _col_view(b1_d.ap(), 0, 16))
            vb_bc = consts.tile([128, DIM], F32, name="vb_bc")
            nc.sync.dma_start(vb_bc[:], _bcast_row(vb_d.ap(), 0, DIM))
            projb_bc = consts.tile([128, DIM], F32, name="projb_bc")
            nc.sync.dma_start(projb_bc[:], _bcast_row(projb_d.ap(), 0, DIM))
            b2_bc = consts.tile([128, DIM], F32, name="b2_bc")
            nc.sync.dma_start(b2_bc[:], _bcast_row(b2_d.ap(), 0, DIM))
            eps_t = consts.tile([128, 1], F32, name="eps_t")
            nc.vector.memset(eps_t[:], EPS)
            ones_bf = consts.tile([1, 64], BF16, name="ones_bf")
            nc.vector.memset(ones_bf[:], 1.0)

            stages = {}   # g -> dict of tiles
            e_tiles = {}  # (h, j) -> E tile
            pref_av = {}  # (gp, h) -> P_av tile allocated early in the bubble
            av_done = set()  # (gp, w, h) AV accumulations already emitted

            def qkv_ln(g):
                """x load + LN1 + transpose + modulate -> y16 (no matmuls).

                Emitted one group early so its ACT sqrt isn't queued behind
                the attention exps and PE always has QKV work ready."""
                st = {}
                xts, xpbs = [], []
                mv = tp.tile([128, 4, 2], F32, name=f"mv1_{g}", tag="mv1")
                for t in range(4):
                    xt = xp.tile([128, DIM], F32, name=f"x_{g}_{t}", tag=f"x{t}")
                    nc.sync.dma_start(xt[:], x_d[(g * 4 + t) * 128:(g * 4 + t + 1) * 128, :])
                    stats = tp.tile([128, 6], F32, name=f"st_{g}_{t}", tag=f"st{t}")
                    nc.vector.bn_stats(stats[:], xt[:])
                    nc.vector.bn_aggr(mv[:, t:t + 1, :], stats[:])
                    xts.append(xt)
                std = tp.tile([128, 4], F32, name=f"sd_{g}", tag="sd1")
                nc.scalar.activation(std[:], mv[:, :, 1:2], AF.Sqrt, bias=eps_t[:])
                rs = tp.tile([128, 4], F32, name=f"rs_{g}", tag="rs1")
                nc.vector.reciprocal(rs[:], std[:])
                xhT = xhtp.tile([128, 4, GRP], F16, name=f"xhT_{g}", tag="xhT")
                for t in range(4):
                    xh = xhp.tile([128, DIM], F16, name=f"xh_{g}_{t}", tag=f"xh{t}")
                    nc.vector.tensor_scalar(xh[:], xts[t][:], mv[:, t:t + 1, 0:1], rs[:, t:t + 1],
                                            op0=AL.subtract, op1=AL.mult)
                    # x + projb precomputed (fp16) so x tiles die early
                    xpb = xpbp.tile([128, DIM], F16, name=f"xpb_{g}_{t}", tag=f"xpb{t}")
                    nc.vector.tensor_tensor(xpb[:], xts[t][:], projb_bc[:], op=AL.add)
                    xpbs.append(xpb)
                    # one batched transpose per token tile (sync ring)
                    nc.sync.dma_start_transpose(xhT[:, :, t * 128:(t + 1) * 128], xh[:])
                st["xpb"] = xpbs
                # modulate per chunk (arow/crow are per-partition on transposed side)
                y16 = ytp.tile([128, 4, GRP], F16, name=f"y16_{g}", tag="y16")
                for c in range(4):
                    nc.vector.tensor_scalar(y16[:, c, :], xhT[:, c, :],
                                            arow_col[:, c:c + 1], crow_col[:, c:c + 1],
                                            op0=AL.mult, op1=AL.add)
                st["y16"] = y16
                st["q"] = [None] * 4
                st["k"] = [None] * 4
                st["v"] = [None] * 4
                return st

            def qkv_piece_qk(g, m):
                """One QK output chunk for group g (4 MMs + bias)."""
                st = stages[g]
                P = ps_g.tile([128, GRP], F32, name=f"Pqk_{g}_{m}", tag="gemm")
                for c in range(4):
                    nc.tensor.matmul(P[:], qkvw_sb[:, c, m * 128:(m + 1) * 128],
                                     st["y16"][:, c, :], start=(c == 0), stop=(c == 3))
                pool = qp if m < 4 else kp
                nm = f"q_{g}_{m}" if m < 4 else f"k_{g}_{m-4}"
                tg = f"q{m}" if m < 4 else f"k{m-4}"
                sb = pool.tile([128, GRP], F16, name=nm, tag=tg)
                nc.vector.tensor_scalar_add(sb[:], P[:], qkb_sb[:, m:m + 1])
                if m < 4:
                    st["q"][m] = sb
                else:
                    st["k"][m - 4] = sb

            def qkv_piece_v(g, t):
                """One V token tile for group g (4 MMs + bias + ones col)."""
                st = stages[g]
                P = ps_g.tile([128, DIM], F32, name=f"Pv_{g}_{t}", tag="gemm")
                for c in range(4):
                    nc.tensor.matmul(P[:], st["y16"][:, c, t * 128:(t + 1) * 128],
                                     qkvw_sb[:, c, 2 * DIM:3 * DIM],
                                     start=(c == 0), stop=(c == 3))
                vt = vp.tile([128, HEADS, HD + 1], F16, name=f"v_{g}_{t}", tag=f"v{t}")
                nc.vector.memset(vt[:, :, HD:HD + 1], 1.0)
                nc.vector.tensor_tensor(
                    vt[:, :, 0:HD],
                    P[:].rearrange("p (h d) -> p h d", h=HEADS),
                    vb_bc[:].rearrange("p (h d) -> p h d", h=HEADS),
                    op=AL.add)
                st["v"][t] = vt

            def make_pieces(g):
                """QKV matmul closures for group g, split into the chunks the
                next attention phase itself needs (main: all QK + V0, ordered
                by first use) and ones deferrable into the LN2 bubble (V1-3)."""
                main = [lambda: qkv_piece_qk(g, 0),
                        lambda: qkv_piece_qk(g, 4),
                        lambda: qkv_piece_v(g, 0),
                        lambda: qkv_piece_qk(g, 1),
                        lambda: qkv_piece_qk(g, 5),
                        lambda: qkv_piece_qk(g, 2),
                        lambda: qkv_piece_qk(g, 6),
                        lambda: qkv_piece_qk(g, 3),
                        lambda: qkv_piece_qk(g, 7)]
                rest = [lambda t=t: qkv_piece_v(g, t) for t in (1, 2, 3)]
                return main, rest

            def sim_exp(gp, h, j):
                """Sim matmul + exp for one (head, key block) -> E tile."""
                hp = h // 2
                off = (h % 2) * 64
                gj, sj = divmod(j, 4)
                qlo = max(0, j - 1)
                qhi = min(nW - 1, j + 1)
                ncols = (qhi - qlo + 1) * WIN
                P_sim = ps_s.tile([128, 384], F32, name=f"Ps_{gp}_{h}_{j}", tag="sim")
                # q columns may span two group tiles -> split segments
                w0 = qlo
                while w0 <= qhi:
                    gq = w0 // 4
                    wend = min(qhi, gq * 4 + 3)
                    c0 = (w0 % 4) * WIN
                    c1 = (wend % 4 + 1) * WIN
                    dst0 = (w0 - qlo) * WIN
                    nc.tensor.matmul(
                        P_sim[:, dst0:dst0 + (c1 - c0)],
                        stages[gj]["k"][hp][off:off + 64, sj * 128:(sj + 1) * 128],
                        stages[gq]["q"][hp][off:off + 64, c0:c1],
                        start=True, stop=True)
                    w0 = wend + 1
                E = ep.tile([128, 384], F16, name=f"E_{gp}_{h}_{j}",
                            tag=f"E{h}_{j % 3}")
                nc.scalar.activation(E[:, 0:ncols], P_sim[:, 0:ncols],
                                     AF.Exp, scale=SIMSCALE)
                e_tiles[(h, j)] = E

            def bubble_fill(gp1, rest_pieces):
                """Work emitted between proj/LN2 and the MLP so the PE queue
                stays busy during the LN2 latency chain: the deferred V pieces
                of group gp1 interleaved with prefetched sims/exps of the
                next attention phase's first head pair (blocks with no
                group-(gp1+1) dependency)."""
                # only blocks whose q windows stay within group gp1
                js = [j for j in range(4 * gp1 + 1, 4 * gp1 + 3) if j < nW]
                work = [(h, j) for j in js for h in (0, 1, 2, 3)]
                for i, pc in enumerate(rest_pieces):
                    pc()
                    for h, j in work[3 * i:3 * i + 3]:
                        sim_exp(gp1, h, j)
                for h, j in work[3 * len(rest_pieces):]:
                    sim_exp(gp1, h, j)
                # head-pair 0's first two AV windows are fully computable here
                for h in (0, 1):
                    pav = ps_a.tile([65, 4, 128], F32, name=f"Pav_{gp1}_{h}", tag="av")
                    pref_av[(gp1, h)] = pav
                    for w in [4 * gp1, 4 * gp1 + 1]:
                        jjs = [jj for jj in (w - 1, w, w + 1) if 0 <= jj < nW]
                        for ji, jj in enumerate(jjs):
                            gjj, sjj = divmod(jj, 4)
                            colofs = (w - max(0, jj - 1)) * WIN
                            nc.tensor.matmul(
                                pav[:, w % 4, :],
                                stages[gjj]["v"][sjj][:, h, :],
                                e_tiles[(h, jj)][:, colofs:colofs + WIN],
                                start=(ji == 0), stop=(ji == len(jjs) - 1))
                        av_done.add((gp1, w, h))

            def attn_stage(gp, pieces):
                # key blocks computed this phase
                j_list = [j for j in range(4 * gp + 1, 4 * gp + 5) if j < nW]
                if gp == 0:
                    j_list = [0] + j_list
                attn_f16 = {}
                a16 = a16p.tile([128, 4, GRP], F16, name=f"a16_{gp}", tag="a16")
                pieces = list(pieces)
                for hp in range(4):
                    heads = (2 * hp, 2 * hp + 1)
                    af = astp.tile([128, GRP], F16, name=f"af_{gp}_{hp}", tag=f"af{hp % 2}")
                    attn_f16[hp] = af
                    P_av = {}
                    for h in heads:
                        if (gp, h) in pref_av:
                            P_av[h] = pref_av.pop((gp, h))
                        else:
                            P_av[h] = ps_a.tile([65, 4, 128], F32, name=f"Pav_{gp}_{h}", tag="av")
                    for j in j_list:
                        for h in heads:
                            if (h, j) not in e_tiles:
                                sim_exp(gp, h, j)
                        # AV for completed windows
                        av_ws = []
                        if 4 * gp <= j - 1 <= 4 * gp + 3:
                            av_ws.append(j - 1)
                        if j == j_list[-1] and j == nW - 1:
                            av_ws.append(nW - 1)
                        for w in av_ws:
                            jjs = [jj for jj in (w - 1, w, w + 1) if 0 <= jj < nW]
                            for h in heads:
                                if (gp, w, h) in av_done:
                                    continue
                                for ji, jj in enumerate(jjs):
                                    gjj, sjj = divmod(jj, 4)
                                    colofs = (w - max(0, jj - 1)) * WIN
                                    nc.tensor.matmul(
                                        P_av[h][:, w % 4, :],
                                        stages[gjj]["v"][sjj][:, h, :],
                                        e_tiles[(h, jj)][:, colofs:colofs + WIN],
                                        start=(ji == 0), stop=(ji == len(jjs) - 1))
                        # interleave one QKV piece of the next group between
                        # attention steps so the PE queue never blocks long
                        if pieces:
                            pieces.pop(0)()
                    # evacuate PSUM: unnormalized attn (DVE when aligned) + sums
                    P_rbc = ps_g.tile([128, GRP], F32, name=f"Prbc_{gp}_{hp}", tag="gemm")
                    for h in heads:
                        off = (h % 2) * 64
                        src = P_av[h][0:64, :, :].rearrange("p a b -> p (a b)")
                        if h % 2 == 0:
                            nc.vector.tensor_copy(af[0:64, :], src)
                        else:
                            nc.scalar.activation(af[64:128, :], src, AF.Identity)
                        ss = sp.tile([1, GRP], F32, name=f"ss_{gp}_{h}", tag="ss")
                        nc.scalar.activation(ss[:],
                                             P_av[h][64:65, :, :].rearrange("p a b -> p (a b)"),
                                             AF.Identity)
                        # on-chip softmax denominators: fast reciprocal, cast
                        # bf16, PE ones-outer-product broadcast across the 64
                        # feature partitions of this head
                        rr = sp.tile([1, GRP], F32, name=f"rr_{gp}_{h}", tag="rr")
                        nc.vector.reciprocal_approx_fast(rr[:], ss[:])
                        rrb = sp.tile([1, GRP], BF16, name=f"rrb_{gp}_{h}", tag="rrb")
                        with nc.allow_low_precision(reason="softmax recip bcast bf16"):
                            nc.vector.tensor_copy(rrb[:], rr[:])
                        nc.tensor.matmul(P_rbc[off:off + 64, :], ones_bf[:], rrb[:],
                                         start=True, stop=True)
                    nc.vector.tensor_tensor(a16[:, hp, :], attn_f16[hp][:], P_rbc[:],
                                            op=AL.mult)
                for pc in pieces:
                    pc()
                return a16

            def proj_ln2(gp, a16):
                cur = stages[gp]
                # proj + residual -> x1 (token-major f32)
                x1_t = []
                for t in range(4):
                    P = ps_g.tile([128, DIM], F32, name=f"Ppr_{gp}_{t}", tag="gemm")
                    for c in range(4):
                        nc.tensor.matmul(P[:], a16[:, c, t * 128:(t + 1) * 128],
                                         projw_sb[:, c, :], start=(c == 0), stop=(c == 3))
                    x1 = x1p.tile([128, DIM], F32, name=f"x1_{gp}_{t}", tag=f"x1{t}")
                    nc.vector.tensor_tensor(x1[:], P[:], cur["xpb"][t][:], op=AL.add)
                    x1_t.append(x1)
                # LN2 -> x_hat2 fp16 per tile (un-batched so each transpose
                # launches as soon as its tile's stats are done)
                h2T = h2tp.tile([128, 4, GRP], F16, name=f"h2T_{gp}", tag="h2T")
                for t in range(4):
                    stats = tp.tile([128, 6], F32, name=f"st2_{gp}_{t}", tag=f"st2{t}")
                    nc.vector.bn_stats(stats[:], x1_t[t][:])
                    mv2 = tp.tile([128, 2], F32, name=f"mv2_{gp}_{t}", tag=f"mv2{t}")
                    nc.vector.bn_aggr(mv2[:], stats[:])
                    std2 = tp.tile([128, 1], F32, name=f"sd2_{gp}_{t}", tag=f"sd2{t}")
                    nc.scalar.activation(std2[:], mv2[:, 1:2], AF.Sqrt, bias=eps_t[:])
                    rs2 = tp.tile([128, 1], F32, name=f"rs2_{gp}_{t}", tag=f"rs2{t}")
                    nc.vector.reciprocal(rs2[:], std2[:])
                    xh2 = xhp.tile([128, DIM], F16, name=f"xh2_{gp}_{t}", tag=f"xh2{t}")
                    nc.vector.tensor_scalar(xh2[:], x1_t[t][:], mv2[:, 0:1], rs2[:],
                                            op0=AL.subtract, op1=AL.mult)
                    nc.scalar.dma_start_transpose(h2T[:, :, t * 128:(t + 1) * 128], xh2[:])
                return x1_t, h2T

            def mlp_stage(gp, x1_t, h2T):
                # MLP1 + gelu (feature-major) fp16
                gel = gelp.tile([128, 16, GRP], F16, name=f"gel_{gp}", tag="gel")
                for f in range(16):
                    P = ps_g.tile([128, GRP], F32, name=f"Pm1_{gp}_{f}", tag="gemm")
                    for c in range(4):
                        nc.tensor.matmul(P[:], w1_sb[:, c, f * 128:(f + 1) * 128],
                                         h2T[:, c, :], start=(c == 0), stop=(c == 3))
                    nc.scalar.activation(gel[:, f, :], P[:], AF.Gelu, bias=b1_sb[:, f:f + 1])
                # MLP2 + bias + residual -> out (token-major)
                for t in range(4):
                    P = ps_g.tile([128, DIM], F32, name=f"Pm2_{gp}_{t}", tag="gemm")
                    for f in range(16):
                        nc.tensor.matmul(P[:], gel[:, f, t * 128:(t + 1) * 128],
                                         w2_sb[:, f, :], start=(f == 0), stop=(f == 15))
                    x1b = tp.tile([128, DIM], F32, name=f"x1b_{gp}_{t}", tag="x1b")
                    nc.vector.tensor_tensor(x1b[:], x1_t[t][:], b2_bc[:], op=AL.add)
                    ot = op.tile([128, DIM], F32, name=f"o_{gp}_{t}", tag="o")
                    nc.vector.tensor_tensor(ot[:], P[:], x1b[:], op=AL.add)
                    nc.sync.dma_start(out_d[(gp * 4 + t) * 128:(gp * 4 + t + 1) * 128, :], ot[:])

            stages[0] = qkv_ln(0)
            if n_groups > 1:
                stages[1] = qkv_ln(1)
            main0, rest0 = make_pieces(0)
            for pc in main0 + rest0:
                pc()
            for gp in range(n_groups):
                if gp + 1 < n_groups:
                    pieces, rest = make_pieces(gp + 1)
                else:
                    pieces, rest = [], []
                a16 = attn_stage(gp, pieces)
                x1_t, h2T = proj_ln2(gp, a16)
                if gp + 2 < n_groups:
                    stages[gp + 2] = qkv_ln(gp + 2)
                if gp + 1 < n_groups:
                    bubble_fill(gp + 1, rest)
                mlp_stage(gp, x1_t, h2T)

    nc.compile()
    return nc


_cache = {}


def _get_nc(n_tok):
    if n_tok not in _cache:
        _cache[n_tok] = build(n_tok)
    return _cache[n_tok]


def _prep_in_maps(inputs):
    return _prep(**inputs)


def _w16(w, chunks):
    """[K, M] f32 -> [128, K//128, M] fp16."""
    K, M = w.shape
    assert K == 128 * chunks
    return np.ascontiguousarray(
        w.astype(np.float16).reshape(chunks, 128, M).transpose(1, 0, 2))


def _prep(x, t_emb, ln1_g, ln1_b, qkv_w, qkv_b, proj_w, proj_b,
          ln2_g, ln2_b, mlp_w1, mlp_b1, mlp_w2, mlp_b2, time_w, time_b):
    x = np.asarray(x, dtype=np.float32)
    t_emb = np.asarray(t_emb, np.float32)
    # host: modulation rows (tiny), fold ln1 gamma/beta
    s = t_emb / (1.0 + np.exp(-t_emb))           # silu
    ss = s @ np.asarray(time_w, np.float32) + np.asarray(time_b, np.float32)
    scale, shift = ss[:, :DIM], ss[:, DIM:]
    g1 = np.asarray(ln1_g, np.float32)
    be1 = np.asarray(ln1_b, np.float32)
    arow = g1[None, :] * (1.0 + scale)                      # [B, 512]
    crow = be1[None, :] * (1.0 + scale) + shift             # [B, 512]
    # fold ln2 gamma/beta into mlp_w1/b1
    g2 = np.asarray(ln2_g, np.float32)
    be2 = np.asarray(ln2_b, np.float32)
    w1f = np.asarray(mlp_w1, np.float32) * g2[:, None]
    b1f = be2 @ np.asarray(mlp_w1, np.float32) + np.asarray(mlp_b1, np.float32)

    qkvw16 = _w16(np.asarray(qkv_w, np.float32), 4)
    projw16 = _w16(np.asarray(proj_w, np.float32), 4)
    w116 = _w16(w1f, 4)
    w216 = _w16(np.asarray(mlp_w2, np.float32), 16)
    qkvb = np.asarray(qkv_b, np.float32)
    qkb = np.ascontiguousarray(qkvb[:2 * DIM])
    vb = np.ascontiguousarray(qkvb[2 * DIM:])
    projb = np.asarray(proj_b, np.float32)
    b2 = np.asarray(mlp_b2, np.float32)

    in_maps = []
    nb = x.shape[0]
    for b in range(nb):
        in_maps.append({
            "x": np.ascontiguousarray(x[b]),
            "arow": np.ascontiguousarray(arow[b]),
            "crow": np.ascontiguousarray(crow[b]),
            "qkvw": qkvw16, "qkb": qkb, "vb": vb,
            "projw": projw16, "projb": projb,
            "w1": w116, "b1": b1f, "w2": w216, "b2": b2,
        })
    return in_maps


def kernel(**inputs):
    in_maps = _prep_in_maps(inputs)
    n_tok = in_maps[0]["x"].shape[0]
    nc = _get_nc(n_tok)
    nb = len(in_maps)
    res = bass_utils.run_bass_kernel_spmd(nc, in_maps, core_ids=list(range(nb)))
    out = np.stack([res.results[b]["out"] for b in range(nb)], axis=0)
    return out
